# revision 5
# baseline (speedup 1.0000x reference)
"""Trainium2 Bass kernel: 1D Winograd F(2,3) along x, fp16 GEMM operands.

Per output-col pair: 4 transform points x 3 dy taps = 6 PE rows/output
vs 9 direct -> 1.5x fewer PE rows.  Weights transformed on host (G23),
BN folded on host, xm pre-scaled by alpha on host, xt/xm uploaded fp16.
Upsample (bilinear 32->64 align-corners) on DVE in fp16, phase-split
(even/odd cols) on Act, x-transform (4 tensor ops) on DVE, GEMM fp16
accumulating in PSUM over (ci,t,dy), M-drain on Act, output transform
(A23) on Pool, BN+ReLU+phase-scatter on Act.
"""
import sys

if '/opt/trn_rl_repo' not in sys.path:
    sys.path.insert(0, '/opt/trn_rl_repo')

import numpy as np
import concourse.bacc as bacc
import concourse.mybir as mybir
from concourse.tile import TileContext
from concourse.bass_utils import run_bass_kernel_spmd

F32 = mybir.dt.float32
F16 = mybir.dt.float16
ALU = mybir.AluOpType
ACTF = mybir.ActivationFunctionType
EPS = 1e-5
N_CORES = 8
EOW = 70          # EO/ypadQ row width: [z z | O(2:34) | z z | E(36:68) | z z]


def _v2(ap2d, offset, rows, rowstep, cols):
    sl = ap2d[:, offset: offset + rows * rowstep]
    return sl.rearrange("p (r c) -> p r c", c=rowstep)[:, :, 0:cols]


def build_patterns():
    k = np.arange(1, 32)
    ko = np.arange(0, 31)
    wxe = -(k / 63.0)
    wxo = (31 - ko) / 63.0
    return np.concatenate([wxe, wxo]).astype(np.float16)


def build_ypat():
    # [q, par, row(8)] y-interp weights (expanded to 64 cols on device)
    out = np.zeros((4, 2, 8), np.float32)
    for q in range(4):
        j0, j1 = 8 * q, 8 * q + 8
        jlo = max(j0, 1)
        for i, j in enumerate(range(jlo, j1)):
            out[q, 0, i] = -j / 63.0             # wye[j-1]
        jhi = min(j1, 31)
        for i, j in enumerate(range(j0, jhi)):
            out[q, 1, i] = (31 - j) / 63.0       # wyo[j]
    return out.reshape(64).astype(np.float16)


def build_nc(skip_feed=False):
    nc = bacc.Bacc(None, target_bir_lowering=True)

    xt_d = nc.dram_tensor("xt", [512, 1024], F16, kind="ExternalInput")
    xm_d = nc.dram_tensor("xm", [512, 4096], F16, kind="ExternalInput")
    pat_d = nc.dram_tensor("pat", [62], F16, kind="ExternalInput")
    ypat_d = nc.dram_tensor("ypat", [64], F16, kind="ExternalInput")
    w0_d = nc.dram_tensor("w0t", [16 * 128, 12 * 128], F16, kind="ExternalInput")
    w1_d = nc.dram_tensor("w1t", [8 * 128, 12 * 128], F16, kind="ExternalInput")
    bn_d = {}
    for nm in ("sc0", "sh0"):
        bn_d[nm] = nc.dram_tensor(nm, [512, 1], F32, kind="ExternalInput")
    for nm in ("sc1", "sh1"):
        bn_d[nm] = nc.dram_tensor(nm, [256, 1], F32, kind="ExternalInput")
    out_d = nc.dram_tensor("out", [256, 4096], F32, kind="ExternalOutput")

    with TileContext(nc) as tc:
        with tc.tile_pool(name="main", bufs=1) as P, \
             tc.tile_pool(name="wp", bufs=8) as WP, \
             tc.tile_pool(name="xmp", bufs=2) as XMP, \
             tc.tile_pool(name="xtp", bufs=2) as XTP, \
             tc.tile_pool(name="msb", bufs=1) as MSB, \
             tc.tile_pool(name="rpl", bufs=1) as RPL, \
             tc.tile_pool(name="outp", bufs=1) as OUTP, \
             tc.tile_pool(name="psum", bufs=2, space="PSUM") as PS:

            # ---- weight slot prefetch: conv0 q0/q1, all ci ----
            wsl0 = {}
            first_feed = {}
            for q in (0, 1):
                for ci in range(4):
                    wt = WP.tile([128, 12 * 128], F16, tag="w",
                                 name=f"w0_{q}_{ci}")
                    nc.sync.dma_start(wt[:], w0_d[(q * 4 + ci) * 128:
                                                  (q * 4 + ci) * 128 + 128, :])
                    wsl0[(q, ci)] = wt
                    if q == 0 and ci == 0:
                        xt00 = XTP.tile([128, 352], F16, tag="xtq", name="xt00")
                        nc.sync.dma_start(xt00[:, 0:9 * 32], xt_d[0:128, 0:9 * 32])
                        xm00 = XMP.tile([128, 1088], F16, tag="xm", name="xm00")
                        nc.sync.dma_start(xm00[:, 0:1024], xm_d[0:128, 0:1024])
                        first_feed = {"xt": xt00, "xm": xm00}

            # persistent planes
            xpadQ = [P.tile([128, 1088], F16, tag=f"xq{i}", name=f"xq{i}") for i in range(4)]
            EO = [P.tile([128, 16 * EOW + 64], F16, tag=f"eo{i}", name=f"eo{i}") for i in range(4)]
            ypadQ = [P.tile([128, 16 * EOW + 64], F16, tag=f"yq{i}", name=f"yq{i}") for i in range(4)]
            XT0 = [P.tile([128, 64 * 128 + 128], F16, tag=f"x0_{i}", name=f"x0_{i}") for i in range(4)]
            XT1 = [P.tile([128, 64 * 128 + 128], F16, tag=f"x1_{i}", name=f"x1_{i}") for i in range(4)]

            # PE warmup on zeroed xpadQ[0]
            nc.gpsimd.memset(xpadQ[0][:].bitcast(F32), 0.0)
            pw = PS.tile([128, 2048], F32, tag="cpsum", name="pwarm")
            for wi in range(48):
                nc.tensor.matmul(pw[:, 0:512], xpadQ[0][:, 0:128],
                                 xpadQ[0][:, 128:640],
                                 start=True, stop=True, skip_group_check=True)

            # zero pad cols of EO / ypadQ (cols 0,1 / 34,35 / 68,69)
            for t_ in EO + ypadQ:
                tv = t_[:, 0:16 * EOW].bitcast(F32).rearrange("p (r c) -> p r c", c=EOW // 2)
                nc.gpsimd.memset(tv[:, :, 0:1], 0.0)
                nc.gpsimd.memset(tv[:, :, 17:18], 0.0)
                nc.gpsimd.memset(tv[:, :, 34:35], 0.0)

            pat = P.tile([128, 62], F16, tag="pat")
            nc.sync.dma_start(pat[:], pat_d[:].partition_broadcast(128))
            ypat_s = P.tile([128, 64], F16, tag="ypat_s")
            nc.sync.dma_start(ypat_s[:], ypat_d[:].partition_broadcast(128))
            ypat = P.tile([128, 1024], F16, tag="ypat")

            def expand_ypat(c):
                # [8] -> [8,64] per parity, reused across the 4 ci of quarter c
                for par in range(2):
                    src = ypat_s[:, (c * 2 + par) * 8:(c * 2 + par) * 8 + 8] \
                        .unsqueeze(2).broadcast_to((128, 8, 64))
                    nc.vector.tensor_copy(
                        _v2(ypat, par * 512, 8, 64, 64), src)
            bn = {}
            for nm, n_cot in (("sc0", 4), ("sh0", 4), ("sc1", 2), ("sh1", 2)):
                for q in range(n_cot):
                    t_ = P.tile([128, 1], F32, tag=f"{nm}_{q}", name=f"{nm}_{q}")
                    nc.sync.dma_start(t_[:], bn_d[nm][q * 128:(q + 1) * 128, :])
                    bn[(nm, q)] = t_

            # ---------------- upsample + fuse (one quarter, one ci) --------
            def us_quarter(ct, j0, alt_pat=False):
                c = j0 // 8
                j1 = j0 + 8
                jstart = max(j0 - 1, 0)
                jstop2 = min(j1 + 1, 32)
                ny = jstop2 - jstart
                if j0 == 0 and ct == 0:
                    xt_t = first_feed["xt"]
                    xm_sb = first_feed["xm"]
                else:
                    xt_t = XTP.tile([128, 352], F16, tag="xtq")
                    nc.sync.dma_start(
                        xt_t[:, 0:ny * 32],
                        xt_d[ct * 128:(ct + 1) * 128,
                             jstart * 32: jstart * 32 + ny * 32])
                    xm_sb = XMP.tile([128, 1088], F16, tag="xm")
                    nc.sync.dma_start(
                        xm_sb[:, 0:1024],
                        xm_d[ct * 128:(ct + 1) * 128,
                             2 * j0 * 64: 2 * j0 * 64 + 1024])
                eng = nc.vector
                d = P.tile([128, 310], F16, tag="dtmp")
                dv = d[:, 0:ny * 31].rearrange("p (y k) -> p y k", k=31)
                eng.tensor_sub(dv, _v2(xt_t, 1, ny, 32, 31),
                               _v2(xt_t, 0, ny, 32, 31))
                xh = P.tile([128, 640], F16, tag="xh")
                xh4 = xh[:, 0:ny * 64].rearrange("p (y k t) -> p y k t",
                                                 k=32, t=2)
                tx = P.tile([128, 310], F16, tag="ttmp")
                txv = tx[:, 0:ny * 31].rearrange("p (y k) -> p y k", k=31)
                pxe = pat[:, 0:31].unsqueeze(1).broadcast_to((128, ny, 31))
                eng.tensor_mul(txv, dv, pxe)
                eng.tensor_add(xh4[:, :, 1:32, 0:1].squeeze(),
                               _v2(xt_t, 1, ny, 32, 31), txv)
                eng.tensor_copy(xh4[:, :, 0:1, 0:1].squeeze(),
                                _v2(xt_t, 0, ny, 32, 1).squeeze())
                pxo = pat[:, 31:62].unsqueeze(1).broadcast_to((128, ny, 31))
                eng.tensor_mul(txv, dv, pxo)
                eng.tensor_add(xh4[:, :, 0:31, 1:2].squeeze(),
                               _v2(xt_t, 0, ny, 32, 31), txv)
                eng.tensor_copy(xh4[:, :, 31:32, 1:2].squeeze(),
                                _v2(xt_t, 31, ny, 32, 1).squeeze())
                nD = ny - 1
                D = P.tile([128, 576], F16, tag="Dtmp")
                eng.tensor_sub(D[:, 0:nD * 64], xh[:, 64:64 + nD * 64],
                               xh[:, 0:nD * 64])
                ty = P.tile([128, 512], F16, tag="tytmp")
                # even rows Y=2j
                jlo = max(j0, 1)
                n = j1 - jlo
                tyv = ty[:, 0:n * 64].rearrange("p (r c) -> p r c", c=64)
                ypt = ypat2 if alt_pat else ypat
                pyev = _v2(ypt, 0, n, 64, 64)
                eng.tensor_mul(tyv, _v2(D, (jlo - 1 - jstart) * 64, n, 64, 64),
                               pyev)
                eng.tensor_add(tyv, _v2(xm_sb, (2 * jlo - 2 * j0) * 64,
                                        n, 128, 64), tyv)
                eng.tensor_add(_v2(xpadQ[ct], (2 * jlo - 16 * c) * 64,
                                   n, 128, 64),
                               _v2(xh, (jlo - jstart) * 64, n, 64, 64), tyv)
                if j0 == 0:
                    eng.tensor_add(xpadQ[ct][:, 0:64], xm_sb[:, 0:64],
                                   xh[:, 0:64])
                # odd rows Y=2j+1
                jhi = min(j1, 31)
                n2 = jhi - j0
                tyv = ty[:, 0:n2 * 64].rearrange("p (r c) -> p r c", c=64)
                pyov = _v2(ypt, 512, n2, 64, 64)
                eng.tensor_mul(tyv, _v2(D, (j0 - jstart) * 64, n2, 64, 64),
                               pyov)
                eng.tensor_add(tyv, _v2(xm_sb, 64, n2, 128, 64), tyv)
                eng.tensor_add(_v2(xpadQ[ct], (2 * j0 + 1 - 16 * c) * 64,
                                   n2, 128, 64),
                               _v2(xh, (j0 - jstart) * 64, n2, 64, 64), tyv)
                if j1 == 32:
                    eng.tensor_add(xpadQ[ct][:, 15 * 64:16 * 64],
                                   xm_sb[:, 15 * 64:16 * 64],
                                   xh[:, (31 - jstart) * 64:
                                      (31 - jstart) * 64 + 64])

            def split_and_xform(ct, c, src, dst):
                # Act: phase-split even/odd cols of 16 spatial rows into EO
                eov = lambda a: _v2(EO[ct], a, 16, EOW, 32)
                sE = src[:, 0:1024].rearrange("p (r k t) -> p r k t", k=32, t=2)
                nc.scalar.activation(eov(36), sE[:, :, :, 0:1].squeeze(),
                                     ACTF.Copy)
                nc.scalar.activation(eov(2), sE[:, :, :, 1:2].squeeze(),
                                     ACTF.Copy)
                # Pool: F(2,3) x-transform -> 4 t-planes of X~ rows 16c..16c+15
                # (Pool is idle during the feed phase; DVE paces conv0)
                xv = lambda t: _v2(dst, (16 * c) * 128 + t * 32, 16, 128, 32)
                nc.gpsimd.tensor_sub(xv(0), eov(1), eov(2))    # d0-d2
                nc.gpsimd.tensor_add(xv(1), eov(36), eov(2))   # d1+d2
                nc.gpsimd.tensor_sub(xv(2), eov(2), eov(36))   # d2-d1
                nc.gpsimd.tensor_sub(xv(3), eov(36), eov(37))  # d1-d3

            # ---------------- conv chunk mms ----------------
            def chunk_mms(XT, wsl, qs, c, ci, y0=None, nr=16):
                # y0: first output row of this sub-window (default whole chunk)
                if y0 is None:
                    y0 = 16 * c
                # windows starting at row -1 lose their first row (zero pad),
                # so order dys to start the psum region with a full window
                dys = (1, 0, 2) if y0 == 0 else (0, 1, 2)
                for q in qs:
                    pt = psum_tiles[(q, c)]
                    for t in range(4):
                        for k, dy in enumerate(dys):
                            lo = y0 - 1 + dy           # first X~ row
                            r0, rows = y0 - 16 * c, nr
                            if lo < 0:
                                lo, r0, rows = 0, r0 + 1, nr - 1
                            elif lo + nr > 64:
                                rows = 64 - lo
                            out = _v2(pt, t * 512 + r0 * 32, rows, 32, 32)
                            rhs = _v2(XT[ci], lo * 128 + t * 32,
                                      rows, 128, 32)
                            nc.tensor.matmul(
                                out, wsl[(q, ci)][:, (t * 3 + dy) * 128:
                                                  (t * 3 + dy) * 128 + 128],
                                rhs,
                                start=(ci == 0 and k == 0),
                                stop=(ci == 3 and k == 2),
                                skip_group_check=True)

            def drain(layer, q, c):
                reng = nc.vector
                pt = psum_tiles.pop((q, c))
                msb = MSB.tile([128, 2048], F16, tag="msb", name=f"m{layer}_{q}_{c}")
                nc.scalar.activation(msb[:], pt[:], ACTF.Copy)
                rpl = RPL.tile([128, 1024], F16, tag="rpl")
                if layer == 0:
                    # rpl = [r1 | r0], accumulated in place
                    reng.tensor_sub(rpl[:, 0:512], msb[:, 512:1024],
                                    msb[:, 1024:1536])
                    reng.tensor_sub(rpl[:, 0:512], rpl[:, 0:512],
                                    msb[:, 1536:2048])
                    reng.tensor_add(rpl[:, 512:1024], msb[:, 0:512],
                                    msb[:, 512:1024])
                    reng.tensor_add(rpl[:, 512:1024], rpl[:, 512:1024],
                                    msb[:, 1024:1536])
                    yv = lambda a: _v2(ypadQ[q], a, 16, EOW, 32)
                    r1v = rpl[:, 0:512].rearrange("p (r c) -> p r c", c=32)
                    r0v = rpl[:, 512:1024].rearrange("p (r c) -> p r c", c=32)
                    nc.scalar.activation(yv(2), r1v, ACTF.Relu,
                                         bias=bn[("sh0", q)][:, 0:1],
                                         scale=bn[("sc0", q)][:, 0:1])
                    nc.scalar.activation(yv(36), r0v, ACTF.Relu,
                                         bias=bn[("sh0", q)][:, 0:1],
                                         scale=bn[("sc0", q)][:, 0:1])
                    # x-transform ypadQ -> XT1 rows
                    eov = lambda a: _v2(ypadQ[q], a, 16, EOW, 32)
                    xv = lambda t: _v2(XT1[q], (16 * c) * 128 + t * 32,
                                       16, 128, 32)
                    nc.vector.tensor_sub(xv(0), eov(1), eov(2))
                    nc.vector.tensor_add(xv(1), eov(36), eov(2))
                    nc.vector.tensor_sub(xv(2), eov(2), eov(36))
                    nc.vector.tensor_sub(xv(3), eov(36), eov(37))
                else:
                    # rpl = [r0 | r1], accumulated in place
                    reng.tensor_add(rpl[:, 0:512], msb[:, 0:512],
                                    msb[:, 512:1024])
                    reng.tensor_add(rpl[:, 0:512], rpl[:, 0:512],
                                    msb[:, 1024:1536])
                    reng.tensor_sub(rpl[:, 512:1024], msb[:, 512:1024],
                                    msb[:, 1024:1536])
                    reng.tensor_sub(rpl[:, 512:1024], rpl[:, 512:1024],
                                    msb[:, 1536:2048])
                    # phase-major DRAM layout [co, r(2), y(64), j(32)];
                    # host de-interleaves x = 2j+r
                    for r in range(2):
                        ob = OUTP.tile([128, 512], F32, tag="ob",
                                       name=f"ob{q}_{c}_{r}")
                        nc.scalar.activation(ob[:], rpl[:, r * 512:
                                                        (r + 1) * 512],
                                             ACTF.Relu,
                                             bias=bn[("sh1", q)][:, 0:1],
                                             scale=bn[("sc1", q)][:, 0:1])
                        nc.sync.dma_start(
                            out_d[q * 128:(q + 1) * 128,
                                  r * 2048 + c * 512: r * 2048 + c * 512 + 512],
                            ob[:])

            psum_tiles = {}

            def open_psum(qs, c, layer):
                for q in qs:
                    psum_tiles[(q, c)] = PS.tile([128, 2048], F32, tag="cpsum",
                                                 name=f"ps{layer}_{q}_{c}")

            # ============ emission ============
            # phase A: quarters 0,1 per ci, chunk0 mms per ci
            open_psum((0, 1), 0, 0)
            ypat2 = P.tile([128, 1024], F16, tag="ypat2")
            expand_ypat(0)
            for par in range(2):
                src = ypat_s[:, (2 + par) * 8:(2 + par) * 8 + 8] \
                    .unsqueeze(2).broadcast_to((128, 8, 64))
                nc.vector.tensor_copy(_v2(ypat2, par * 512, 8, 64, 64), src)
            for ci in range(4):
                if skip_feed:
                    break
                us_quarter(ci, 0)
                split_and_xform(ci, 0, xpadQ[ci], XT0[ci])
            for ci in range(4):
                chunk_mms(XT0, wsl0, (0, 1), 0, ci, y0=0, nr=8)
            for ci in range(4):
                if not skip_feed:
                    us_quarter(ci, 8, alt_pat=True)
                    split_and_xform(ci, 1, xpadQ[ci], XT0[ci])
                chunk_mms(XT0, wsl0, (0, 1), 0, ci, y0=8, nr=8)
            for q in (0, 1):
                drain(0, q, 0)
            # phase B: quarter 2, chunk1
            open_psum((0, 1), 1, 0)
            expand_ypat(2)
            for ci in range(4):
                if not skip_feed:
                    us_quarter(ci, 16)
                    split_and_xform(ci, 2, xpadQ[ci], XT0[ci])
                chunk_mms(XT0, wsl0, (0, 1), 1, ci)
            for q in (0, 1):
                drain(0, q, 1)
            # phase C: quarter 3, chunk2
            open_psum((0, 1), 2, 0)
            expand_ypat(3)
            for ci in range(4):
                if not skip_feed:
                    us_quarter(ci, 24)
                    split_and_xform(ci, 3, xpadQ[ci], XT0[ci])
                chunk_mms(XT0, wsl0, (0, 1), 2, ci)
            for q in (0, 1):
                drain(0, q, 2)
            # chunk3, then pair23 slot loads (after last readers of wsl0)
            open_psum((0, 1), 3, 0)
            for ci in range(4):
                chunk_mms(XT0, wsl0, (0, 1), 3, ci)
            wsl0b = {}
            for q in (2, 3):
                for ci in range(4):
                    wt = WP.tile([128, 12 * 128], F16, tag="w",
                                 name=f"w0_{q}_{ci}")
                    nc.sync.dma_start(wt[:], w0_d[(q * 4 + ci) * 128:
                                                  (q * 4 + ci) * 128 + 128, :])
                    wsl0b[(q, ci)] = wt
            for q in (0, 1):
                drain(0, q, 3)
            # pair23 chunks 0..3
            for c in range(4):
                open_psum((2, 3), c, 0)
                for ci in range(4):
                    chunk_mms(XT0, wsl0b, (2, 3), c, ci)
                for q in (2, 3):
                    drain(0, q, c)
            # conv1 slots
            wsl1 = {}
            for q in (0, 1):
                for ci in range(4):
                    wt = WP.tile([128, 12 * 128], F16, tag="w",
                                 name=f"w1_{q}_{ci}")
                    nc.sync.dma_start(wt[:], w1_d[(q * 4 + ci) * 128:
                                                  (q * 4 + ci) * 128 + 128, :])
                    wsl1[(q, ci)] = wt
            for c in range(4):
                open_psum((0, 1), c, 1)
                for ci in range(4):
                    chunk_mms(XT1, wsl1, (0, 1), c, ci)
                for q in (0, 1):
                    drain(1, q, c)

    nc.finalize()
    return nc


_CACHED_NC = None


def _get_nc():
    global _CACHED_NC
    if _CACHED_NC is None:
        _CACHED_NC = build_nc()
    return _CACHED_NC


G23 = np.array([[1, 0, 0], [.5, .5, .5], [.5, -.5, .5], [0, 0, 1]])


def packw(w, n_cot):
    wt = np.einsum('tk,ocdk->ocdt', G23, w.astype(np.float64))
    a = wt.transpose(1, 3, 2, 0).reshape(4, 128, 4, 3, n_cot, 128)
    a = a.transpose(4, 0, 1, 2, 3, 5)
    return np.ascontiguousarray(a).astype(np.float16).reshape(
        n_cot * 4 * 128, 12 * 128)


def kernel(**inputs) -> np.ndarray:
    xt = np.asarray(inputs["xt"], np.float32)      # [8,512,32,32]
    xm = np.asarray(inputs["xm"], np.float32)      # [8,512,64,64]
    alpha = float(np.asarray(inputs["alpha"], np.float32).reshape(1)[0])
    w0 = np.asarray(inputs["w0"], np.float32)
    w1 = np.asarray(inputs["w1"], np.float32)

    def bnfold(g, b, m, v):
        sc = g / np.sqrt(v + EPS)
        sh = b - m * sc
        return sc.astype(np.float32), sh.astype(np.float32)

    sc0, sh0 = bnfold(*[np.asarray(inputs[k], np.float32)
                        for k in ("g0", "b0", "m0", "v0")])
    sc1, sh1 = bnfold(*[np.asarray(inputs[k], np.float32)
                        for k in ("g1", "b1", "m1", "v1")])

    common = {
        "pat": build_patterns(), "ypat": build_ypat(),
        "w0t": packw(w0, 4), "w1t": packw(w1, 2),
        "sc0": sc0.reshape(512, 1), "sh0": sh0.reshape(512, 1),
        "sc1": sc1.reshape(256, 1), "sh1": sh1.reshape(256, 1),
    }
    xm_s = (alpha * xm).astype(np.float16)
    xt_h = xt.astype(np.float16)
    in_maps = []
    for b in range(N_CORES):
        m = dict(common)
        m["xt"] = np.ascontiguousarray(xt_h[b].reshape(512, 1024))
        m["xm"] = np.ascontiguousarray(xm_s[b].reshape(512, 4096))
        in_maps.append(m)

    nc = _get_nc()
    res = run_bass_kernel_spmd(nc, in_maps, core_ids=list(range(N_CORES)))
    out = np.empty((N_CORES, 256, 64, 64), np.float32)
    for b in range(N_CORES):
        arr = res.results[b]["out"].reshape(256, 2, 64, 32)
        out[b, :, :, 0::2] = arr[:, 0]
        out[b, :, :, 1::2] = arr[:, 1]
    return out
